# revision 1
# baseline (speedup 1.0000x reference)
"""Trainium2 Bass kernel for tanh-attention (nn_Attention_50362786513376).

reference:
  q = (x @ Wq.T) * dk^-0.5 ; k = x @ Wk.T ; v = x        (heads = 8, dk = 64)
  out = tanh(q k^T) v   per (batch, head),  merged back to [b, n, dim]

Sharding: 8 cores = 4 batches x 2 head-halves (4 heads per core).
Host pre-work (free, exact): transpose x[b] -> xT, slice v channels, slice +
scale + transpose weights. Device per core:
  Q^T = WqT.T @ xT, K^T = WkT.T @ xT     (f16; all projections upfront,
                                          ct-outer pairs chasing the xT DMA)
  per head pair p, i-quarter, j-tile: S^T[j,i] = K^T.T Q^T (row-packed pairs)
  tanh on ScalarE PSUM->SBUF (the throughput bottleneck: n^2*h*b/8 elements)
  out^T[d,i] += v[j,:].T @ tanh(S^T)     (accumulated in PSUM over j)
Host post-work: out[b,:,half] = outT.T
"""
import numpy as np

HEADS = 8
DK = 64
B = 4
N = 2048
DIM = 512
SCALE = DK ** (-0.5)
NCORES = 8
HALF = DIM // 2  # 256 channels per core (4 heads)

_built = None
_built_cfg = None
PROJ_DTYPE = "f16"   # "f32r" | "f16"  (x / weights / projection matmuls)
ATTN_DTYPE = "f16"    # "f32r" | "f16"  (Q^T/K^T, qk mms)
V_DTYPE = "f16"       # "f16" | "bf16"  (tanh output + v operand of the AV mms)
TRACE = False
TRACE_KW = {}


def _build():
    from contextlib import ExitStack

    import concourse.tile as tile
    from concourse import bacc, mybir

    F32 = mybir.dt.float32
    DT = {"f32r": mybir.dt.float32r, "f16": mybir.dt.float16,
          "bf16": mybir.dt.bfloat16}
    PROJ_DT = DT[PROJ_DTYPE]
    ATTN_DT = DT[ATTN_DTYPE]
    V_DT = DT[V_DTYPE]
    Tanh = mybir.ActivationFunctionType.Tanh

    nc = bacc.Bacc("TRN2", target_bir_lowering=False, debug=False,
                   num_devices=NCORES)
    xT_ap = nc.dram_tensor("xT", [DIM, N], PROJ_DT, kind="ExternalInput").ap()
    xv_ap = nc.dram_tensor("xv", [N, HALF], V_DT, kind="ExternalInput").ap()
    wqT_ap = nc.dram_tensor("wqT", [DIM, HALF], PROJ_DT,
                            kind="ExternalInput").ap()
    wkT_ap = nc.dram_tensor("wkT", [DIM, HALF], PROJ_DT,
                            kind="ExternalInput").ap()
    outT_ap = nc.dram_tensor("outT", [HALF, N], F32, kind="ExternalOutput").ap()

    NT = N // 512          # 4 t-chunks of 512
    NJ = N // 128          # 16 j-tiles

    with tile.TileContext(nc) as tc:
        with ExitStack() as ctx:
            const = ctx.enter_context(tc.tile_pool(name="const", bufs=1))
            qk_pool = ctx.enter_context(tc.tile_pool(name="qk", bufs=1))
            tanh_pool = ctx.enter_context(tc.tile_pool(name="tanh", bufs=6))
            stg_pool = ctx.enter_context(tc.tile_pool(name="stg", bufs=6))

            # ---- load inputs (xT on sync queue: projections chase its
            # chunks; weights + xv on the scalar queue in parallel) ----
            xT_sb = const.tile([128, 4 * N], PROJ_DT)
            wq_sb = const.tile([128, 4 * HALF], PROJ_DT)
            wk_sb = const.tile([128, 4 * HALF], PROJ_DT)
            for ct in range(4):
                nc.sync.dma_start(xT_sb[:, ct * N:ct * N + 1024],
                                  xT_ap[ct * 128:(ct + 1) * 128, 0:1024])
            for w_sb, w_ap in ((wk_sb, wkT_ap), (wq_sb, wqT_ap)):
                for ct in range(4):
                    nc.scalar.dma_start(w_sb[:, ct * HALF:(ct + 1) * HALF],
                                        w_ap[ct * 128:(ct + 1) * 128, :])
            for ct in range(4):
                nc.scalar.dma_start(xT_sb[:, ct * N + 1024:ct * N + 2048],
                                    xT_ap[ct * 128:(ct + 1) * 128, 1024:2048])
            # xv [2048, 256] -> one tile, 16 contiguous j-slice DMAs
            xv_sb = const.tile([128, 16 * HALF], V_DT)
            for j in range(16):
                nc.scalar.dma_start(xv_sb[:, j * HALF:(j + 1) * HALF],
                                    xv_ap[j * 128:(j + 1) * 128, :])

            # ---- projections + attention ----
            # PSUM: ps_S 3 bufs x [128,1024] (6 banks) + ps_acc 2 x [64,512]
            # (2 banks). Projection groups borrow ps_S/ps_acc slots (both
            # idle until the attention stream starts).
            QT = [qk_pool.tile([128, N], ATTN_DT, tag=f"qt{p}", name=f"qt{p}")
                  for p in range(2)]
            KT = [qk_pool.tile([128, N], ATTN_DT, tag=f"kt{p}", name=f"kt{p}")
                  for p in range(2)]
            ps_S = ctx.enter_context(
                tc.tile_pool(name="ps_S", bufs=3, space="PSUM"))
            ps_acc = ctx.enter_context(
                tc.tile_pool(name="ps_acc", bufs=2, space="PSUM"))

            # all projections upfront: groups ct-outer in pairs so the
            # first matmuls chase the chunked xT DMA; 3 S-slots pipeline
            # group pairs against their DVE copies
            def proj_pair(dst, w_sb, p, t4_pair):
                # second tile borrows an (idle-during-proj) acc bank so the
                # three S slots keep a free slot for rotation
                ps2 = {t4_pair[0]: ps_S.tile([128, 512], F32, tag="S",
                                             name="proj_ps"),
                       t4_pair[1]: ps_acc.tile([128, 512], F32, tag="acc",
                                               name="proj_ps2")}
                for ct in range(4):
                    lhsT = w_sb[:, ct * HALF + p * 128:
                                ct * HALF + (p + 1) * 128]
                    for t4 in t4_pair:
                        rhs = xT_sb[:, ct * N + t4 * 512:
                                    ct * N + t4 * 512 + 512]
                        nc.tensor.matmul(ps2[t4][:], lhsT, rhs,
                                         start=(ct == 0), stop=(ct == 3))
                for t4 in t4_pair:
                    nc.vector.tensor_copy(dst[p][:, t4 * 512:(t4 + 1) * 512],
                                          ps2[t4])

            for t4_pair in ((0, 1), (2, 3)):
                for p in range(2):
                    for dst, w_sb in ((KT, wk_sb), (QT, wq_sb)):
                        proj_pair(dst, w_sb, p, t4_pair)

            for p in range(2):
                for iq in range(4):          # i-quarter: i cols iq*512..+512
                    acc = [ps_acc.tile([64, 512], F32, tag="acc", name="acc")
                           for par in range(2)]
                    i0 = iq * 512
                    for j in range(NJ):
                        S = ps_S.tile([128, 1024], F32, tag="S", name="S")
                        # row-packed pair: head parity 0 on PE rows 0-63,
                        # parity 1 on rows 64-127
                        nc.tensor.matmul(
                            S[:, 0:512],
                            KT[p][0:64, j * 128:(j + 1) * 128],
                            QT[p][0:64, i0:i0 + 512],
                            start=True, stop=True, tile_position=(0, 0))
                        nc.tensor.matmul(
                            S[:, 512:1024],
                            KT[p][64:128, j * 128:(j + 1) * 128],
                            QT[p][64:128, i0:i0 + 512],
                            start=True, stop=True, tile_position=(64, 0))
                        T = tanh_pool.tile([128, 1024], V_DT, tag="T",
                                           name="T")
                        nc.scalar.activation(T[:], S[:], Tanh)
                        for par in range(2):
                            lh = 2 * p + par
                            v = xv_sb[:, j * HALF + lh * 64:
                                      j * HALF + lh * 64 + 64]
                            nc.tensor.matmul(
                                acc[par][:],
                                v,
                                T[:, par * 512:(par + 1) * 512],
                                start=(j == 0), stop=(j == NJ - 1))
                    for par in range(2):
                        lh = 2 * p + par
                        st = stg_pool.tile([64, 512], F32, tag="stg",
                                           name="stg")
                        nc.vector.tensor_copy(st[:], acc[par][:])
                        nc.sync.dma_start(
                            outT_ap[lh * 64:(lh + 1) * 64,
                                    iq * 512:(iq + 1) * 512],
                            st[:])

    nc.compile()
    return nc


def _get_built():
    global _built, _built_cfg
    cfg = (PROJ_DTYPE, ATTN_DTYPE, V_DTYPE)
    if _built is None or _built_cfg != cfg:
        _built = _build()
        _built_cfg = cfg
    return _built


def kernel(x, Wq, Wk):
    from concourse.bass_utils import run_bass_kernel_spmd

    x = np.asarray(x, dtype=np.float32)
    Wq = np.asarray(Wq, dtype=np.float32)
    Wk = np.asarray(Wk, dtype=np.float32)

    import ml_dtypes
    proj_np = np.float16 if PROJ_DTYPE == "f16" else np.float32
    v_np = {"f16": np.float16, "bf16": ml_dtypes.bfloat16}[V_DTYPE]

    nc = _get_built()
    in_maps = []
    for c in range(NCORES):
        b, half = c // 2, c % 2
        sl = slice(half * HALF, (half + 1) * HALF)
        in_maps.append({
            "xT": np.ascontiguousarray(x[b].T).astype(proj_np),
            "xv": np.ascontiguousarray(x[b][:, sl]).astype(v_np),
            "wqT": np.ascontiguousarray((SCALE * Wq[sl, :]).T).astype(proj_np),
            "wkT": np.ascontiguousarray(Wk[sl, :].T).astype(proj_np),
        })
    try:
        res = run_bass_kernel_spmd(nc, in_maps, core_ids=list(range(NCORES)),
                                   trace=TRACE, **TRACE_KW)
    except Exception:
        # transient device wedge (NRT_EXEC_UNIT_UNRECOVERABLE) recovers on
        # retry; one attempt is enough in practice
        import time as _time
        _time.sleep(2.0)
        res = run_bass_kernel_spmd(nc, in_maps, core_ids=list(range(NCORES)),
                                   trace=TRACE, **TRACE_KW)
    out = np.empty((B, N, DIM), np.float32)
    for c in range(NCORES):
        b, half = c // 2, c % 2
        out[b, :, half * HALF:(half + 1) * HALF] = res.results[c]["outT"].T
    if TRACE:
        kernel.last_results = res
    return out

